# revision 13
# baseline (speedup 1.0000x reference)
"""3x3 valid conv (cross-correlation) of an 8192x8192 fp32 image on 8 TRN2 NeuronCores.

Strategy
--------
Output rows are sharded across 8 cores: each core computes 8 full 126-row
band blocks (1008 rows), and the leftover 126-row slab (out rows 8064..8189)
is split BY WIDTH across the cores (~1024 cols each). Every core receives its
input rows WITH the 2-row halo included, so no on-device collectives.

The conv runs on the TensorEngine as banded matmuls: for a block of 128 input
rows, out[o, c] += sum_p band_d[p, o] * x[p, c+d], band_d[p, o] = w[p-o, d].
The 3 column taps d=0,1,2 are 3 matmuls over column-shifted views of the same
SBUF tile, accumulated in PSUM (one [126, 512] tile per PSUM bank).

Precision: the tolerance gate is 2e-2, so x and w are rounded to bf16 on the
host and the output is written back as bf16 (measured end-to-end error 4.4e-3
scale-relative: bf16 products are exact in the PE's fp32 accumulation; the
only error is the three roundings). This halves HBM traffic in BOTH
directions vs fp32 (~33.5 MB/core total) and cuts the TensorEngine to 3
matmuls per tile, putting the kernel at the DMA roofline (~330 GB/s/core
measured; a DMA-only probe of the same traffic runs no faster than the full
kernel). PSUM->SBUF (with f32->bf16 convert) is round-robined across the
Scalar and Vector engines (GPSIMD/Pool cannot read PSUM) so neither copy
engine becomes the bottleneck. Input loads are kept 2 units ahead of compute
on the SP DMA queue and each load/store is split in 2 column halves (~8 KB
descriptors measured fastest; 4+-way splits and unsplit 16 KB both slower).
Bias is a scalar; it is added (in f32, exactly as the reference does) on the
host during the gather step.
"""
import numpy as np

H = 8192
W = 8192
OH = H - 2
OW = W - 2
NCORES = 8
BLK_OUT = 126
NBLK = 8  # full band blocks per core
RPC = NBLK * BLK_OUT  # 1008 contiguous output rows per core
IN_ROWS = RPC + 2  # 1010 input rows per core shard
XSTRIDE = 8448  # padded row stride of DRAM shards: != data width (keeps the
# DMA lowering from collapsing rows into giant descriptors) and 512-B aligned
# (16896 B rows; measured ~9% faster than an unaligned 16400-B stride)
WT = 512  # PSUM tile width (one 2KB bank); 8190 = 15*512 + 510
NWT = 16
SPLIT_A = 8 * WT  # 4096: 8192-B-aligned load split (tiles 0..7 need 2 more
# cols for the d=1,2 taps; the tile dep-tracker just adds a loadB dep there)
# leftover slab: out rows [8064, 8190) split by width across cores
SLAB_R0 = NCORES * RPC  # 8064
SLAB_OC = 1024  # slab output cols per core (core 7: only 1022 valid)
SLAB_IC = SLAB_OC + 2
SLAB_STRIDE = 1280  # 2560-B rows, 512-B aligned

_cache = {}


def _build(reps=1, xbufs=3, ybufs=3, nload=2, dualq=False):
    from contextlib import ExitStack

    import concourse.bacc as bacc
    import concourse.tile as tile
    import concourse.mybir as mybir

    f32 = mybir.dt.float32
    bf16 = mybir.dt.bfloat16
    nc = bacc.Bacc("TRN2", target_bir_lowering=False, debug=False)
    xs = nc.dram_tensor("xs", [IN_ROWS, XSTRIDE], bf16, kind="ExternalInput")
    xs2 = nc.dram_tensor("xs2", [128, SLAB_STRIDE], bf16, kind="ExternalInput")
    wb = nc.dram_tensor("wb", [128, 378], bf16, kind="ExternalInput")
    ys = nc.dram_tensor("ys", [RPC, XSTRIDE], bf16, kind="ExternalOutput")
    ys2 = nc.dram_tensor("ys2", [BLK_OUT, SLAB_STRIDE], bf16, kind="ExternalOutput")
    with tile.TileContext(nc) as tc:
        with (
            tc.tile_pool(name="wpool", bufs=1) as wpool,
            tc.tile_pool(name="xraw", bufs=xbufs) as xraw,
            tc.tile_pool(name="yout", bufs=ybufs) as yout,
            tc.tile_pool(name="psum", bufs=8, space="PSUM") as psum,
            ExitStack() as rep_ctx,
        ):
            wt = wpool.tile([128, 378], bf16)
            nc.sync.dma_start(wt[:], wb[:])
            if reps > 1:
                # timing-only variant: repeat the body on-device so per-
                # iteration device time can be isolated from the (large)
                # axon dispatch overhead
                rep_ctx.enter_context(tc.For_i(0, reps, 1))

            copy_engines = [
                lambda o, i: nc.scalar.copy(o, i),
                lambda o, i: nc.vector.tensor_copy(o, i),
            ]
            ci = [0]
            eng_b = nc.scalar if dualq else nc.sync  # queue for B-half DMAs

            # the tiny slab unit goes FIRST: its 0.7us load primes the PE
            # while the first full band block (5.8us load) streams in
            units = [(xs2, 0, ys2, 0, SLAB_OC, 2)] + [
                (xs, j * BLK_OUT, ys, j * BLK_OUT, OW, NWT) for j in range(NBLK)
            ]
            xtiles = {}

            def load(i):
                """Column-split input DMAs so early tiles can start sooner."""
                src, src_row, _, _, ocols, _ = units[i]
                icols = ocols + 2
                xr = xraw.tile([128, W], bf16, tag="xr")
                ca = min(SPLIT_A, icols)
                nc.sync.dma_start(xr[:, :ca], src[src_row : src_row + 128, :ca])
                if icols > ca:
                    eng_b.dma_start(
                        xr[:, ca:icols], src[src_row : src_row + 128, ca:icols]
                    )
                xtiles[i] = xr

            def compute_store(i):
                """3 matmuls per width tile into a PSUM bank, then a
                PSUM->SBUF bf16 copy round-robined across Act/DVE."""
                _, _, dst, dst_row, ocols, ntl = units[i]
                xr = xtiles.pop(i)
                yo = yout.tile([BLK_OUT, OW], bf16, tag="yo")
                for t in range(ntl):
                    w_t = min(WT, ocols - t * WT)
                    pst = psum.tile([BLK_OUT, WT], f32, tag="ps")
                    for d in range(3):
                        nc.tensor.matmul(
                            pst[:, :w_t],
                            wt[:, d * BLK_OUT : d * BLK_OUT + BLK_OUT],
                            xr[:, t * WT + d : t * WT + d + w_t],
                            start=(d == 0),
                            stop=(d == 2),
                        )
                    copy_engines[ci[0] % 2](
                        yo[:, t * WT : t * WT + w_t], pst[:, :w_t]
                    )
                    ci[0] += 1
                cb = min(8 * WT, ocols)
                nc.sync.dma_start(
                    dst[dst_row : dst_row + BLK_OUT, :cb], yo[:, :cb]
                )
                if ocols > cb:
                    eng_b.dma_start(
                        dst[dst_row : dst_row + BLK_OUT, cb:ocols], yo[:, cb:ocols]
                    )

            # software pipeline: keep nload loads in flight ahead of compute
            # so a store waiting at the SP queue head never starves the PE
            for i in range(nload):
                load(i)
            for i in range(len(units)):
                if i + nload < len(units):
                    load(i + nload)
                compute_store(i)
    nc.compile()
    return nc


def _get_nc():
    if "nc" not in _cache:
        _cache["nc"] = _build()
    return _cache["nc"]


def make_inputs(x, weight, bias):
    """Host-side shard/prep: per-core input maps for run_bass_kernel_spmd."""
    import ml_dtypes

    bf16 = ml_dtypes.bfloat16
    xb = np.asarray(x, np.float32).astype(bf16)
    w16 = np.asarray(weight, np.float32).astype(bf16).astype(np.float32)
    wbm = np.zeros((128, 378), np.float32)
    o = np.arange(BLK_OUT)
    for d in range(3):
        for k in range(3):
            wbm[o + k, d * BLK_OUT + o] = w16[k, d]
    wbm = wbm.astype(bf16)
    in_maps = []
    for i in range(NCORES):
        xsh = np.zeros((IN_ROWS, XSTRIDE), bf16)
        xsh[:, :W] = xb[i * RPC : i * RPC + IN_ROWS]
        xs2 = np.zeros((128, SLAB_STRIDE), bf16)
        c0 = i * SLAB_OC
        c1 = min(c0 + SLAB_IC, W)
        xs2[:, : c1 - c0] = xb[SLAB_R0 : SLAB_R0 + 128, c0:c1]
        in_maps.append({"xs": xsh, "xs2": xs2, "wb": wbm})
    return in_maps


def kernel(x, weight, bias):
    from concourse.bass_utils import run_bass_kernel_spmd

    nc = _get_nc()
    in_maps = make_inputs(x, weight, bias)
    res = run_bass_kernel_spmd(nc, in_maps, list(range(NCORES)))
    b0 = np.float32(np.asarray(bias).reshape(-1)[0])
    out = np.empty((OH, OW), np.float32)
    for i in range(NCORES):
        out[i * RPC : (i + 1) * RPC] = (
            res.results[i]["ys"][:, :OW].astype(np.float32) + b0
        )
        c0 = i * SLAB_OC
        c1 = min(c0 + SLAB_OC, OW)
        out[SLAB_R0:OH, c0:c1] = (
            res.results[i]["ys2"][:, : c1 - c0].astype(np.float32) + b0
        )
    return out
